# revision 12
# baseline (speedup 1.0000x reference)
"""Trainium2 Bass kernel for nn_CPCModel_50878182588587 (vq_codebook).

Computes, for inputs encodedData [B,N,D] and protos [K,D]:
  pass1: FCM memberships of v vs protos (p=2), x2 = 0.5*v + 0.5*(belong@protos)
  pass2: FCM memberships of x2 vs protos (p=2)  -> output [B,N,K]

Sharding: data-parallel over B across 8 NeuronCores; protos replicated.

v6 dataflow (T=8192 tokens/core, macro-tiles of 512 tokens):
  - Host pre-transposes the input: xh = 0.5*x^T in bf16 (x2 add, loaded in
    4 token-chunks so compute starts early) and fp8-e4m3 (dist1 matmuls);
    2*||x||^2 (hi/lo bf16 split) is a static row.
  - dist1 mains run as ONE fp8 DoubleRow matmul per k-chunk: the [128,2,*]
    chunk layout maps the 256-deep d-contraction onto a single matmul
    (out += sum_j lhsT[:,j].T @ rhs[:,j]); a 3-row bf16 augmented matmul
    adds 2*||v||^2 (hi/lo) + 2*c2.  The x2 distance scale cancels in the
    membership normalization.
  - dist2 stays bf16 (2 mains + 2-row aug) for accuracy: x2 feeds it as
    bf16 stationary slices.
  - 1/sq via single-op DVE reciprocal_approx_fast; s1-row via ones-matmul;
    isn broadcast via rank-1 matmul; x2 = vth + tg*isn on DVE.
  - pass2 in orientation A: w2=1/sq2 (f32) on DVE; s2 via ACT copy+accum;
    final normalize split 2/2 between ACT (per-partition scale column) and
    DVE tensor_scalar (NO gpsimd -- its tensor_scalar measured 7.5us/tile).
  - Emission is software-pipelined: pass2(i-1) sits between pass1(i) and
    x2norm(i) so the PE stream never starves (keeps HAM warm).
"""

import sys

import numpy as np

sys.path.insert(0, "/opt/trn_rl_repo")

import concourse.bass as bass  # noqa: E402
from concourse import bacc  # noqa: E402
import concourse.mybir as mybir  # noqa: E402
import concourse.tile as tile  # noqa: E402

B, N, D, K = 64, 1024, 256, 512
NCORES = 8
MACRO = 512  # tokens per macro-tile
NCHUNK = 8  # input token-chunks (startup latency)
f32 = mybir.dt.float32
bf16 = mybir.dt.bfloat16
fp8 = mybir.dt.float8e4
FT = mybir.ActivationFunctionType
OP = mybir.AluOpType
DR = mybir.MatmulPerfMode.DoubleRow


def recip_fast(nc, out, in_):
    """reciprocal_approx_fast with any output dtype (wrapper asserts fp32)."""
    from concourse.dve_ops import RECIP_APPROX_FAST_CONSTS, RECIPROCAL_APPROX_FAST

    c = RECIP_APPROX_FAST_CONSTS
    return nc.vector._custom_dve(
        RECIPROCAL_APPROX_FAST, out=out, in0=in_, s0=c["s0"], s1=c["s1"], imm2=c["imm2"]
    )


def build_bass(T, do_compile=True):
    assert T % MACRO == 0 and T % NCHUNK == 0
    nmacro = T // MACRO
    tchunk = T // NCHUNK
    nc = bacc.Bacc(trn_type="TRN2")

    # xh layouts [2*128, T]: chunk c holds x^T rows c*128..c*128+127 (x 0.5)
    xh_d = nc.dram_tensor("xh", [2 * 128, T], bf16, kind="ExternalInput")
    xh8_d = nc.dram_tensor("xh8", [2 * 128, T], fp8, kind="ExternalInput")
    # v2aug rows: [0]=v2hi, [1]=v2lo (hi/lo split of 2*||x||^2), [2]=ones
    v2aug_d = nc.dram_tensor("v2aug", [3, T], bf16, kind="ExternalInput")
    # aug1l rows: [0]=ones, [1]=ones, [2]=2*c2   (pairs with v2aug rows)
    aug1l_d = nc.dram_tensor("aug1l", [3, K], bf16, kind="ExternalInput")
    # dist1 DoubleRow stationary [128, 2, K]: [:,j,:] = chunk j of -8*protos.T
    p8t_d = nc.dram_tensor("p8t", [2 * 128, K], fp8, kind="ExternalInput")
    ptm2_d = nc.dram_tensor("ptm2", [D, K], bf16, kind="ExternalInput")  # -2*protos.T
    pn05_d = nc.dram_tensor("pn05", [K, D], bf16, kind="ExternalInput")  # 0.5*protos
    # aug2r rows: [0]=ones (pairs x2n), [1]=c2
    aug2r_d = nc.dram_tensor("aug2r", [2, K], bf16, kind="ExternalInput")
    rowinit_d = nc.dram_tensor("rowinit", [2, MACRO], bf16, kind="ExternalInput")
    consts_d = nc.dram_tensor("consts", [128, 1], bf16, kind="ExternalInput")  # ones
    onesrow_d = nc.dram_tensor("onesrow", [1, 128], bf16, kind="ExternalInput")
    out_d = nc.dram_tensor("out", [T, K], f32, kind="ExternalOutput")

    with tile.TileContext(nc) as tc:
        with (
            tc.tile_pool(name="singles", bufs=1) as singles,
            tc.tile_pool(name="w1", bufs=8) as w1p,
            tc.tile_pool(name="isn", bufs=2) as isnp,
            tc.tile_pool(name="bcs", bufs=2) as bcsp,
            tc.tile_pool(name="th", bufs=4) as thp,
            tc.tile_pool(name="xt", bufs=4) as xtp,
            tc.tile_pool(name="sqx", bufs=4) as sqxp,
            tc.tile_pool(name="w2", bufs=8) as w2p,
            tc.tile_pool(name="scr", bufs=2) as scrp,
            tc.tile_pool(name="s2c", bufs=2) as s2cp,
            tc.tile_pool(name="r2", bufs=2) as r2p,
            tc.tile_pool(name="ob", bufs=2) as obp,
            tc.tile_pool(name="sq1p", bufs=2, space="PSUM") as sq1_ps,
            tc.tile_pool(name="rwp", bufs=1, space="PSUM") as rows_ps,
            tc.tile_pool(name="bcp", bufs=1, space="PSUM") as bc_ps,
            tc.tile_pool(name="tgp", bufs=2, space="PSUM") as tg_ps,
            tc.tile_pool(name="sq2p", bufs=2, space="PSUM") as sq2_ps,
        ):
            # ---- statics, ordered so macro 0's dependencies land first ----
            xq8 = []
            xq = []
            t8 = singles.tile([128, 2, tchunk], fp8, tag="xq8_0")
            nc.sync.dma_start(
                out=t8, in_=xh8_d[:, 0:tchunk].rearrange("(c p) t -> p c t", p=128)
            )
            xq8.append(t8)
            p8t_sb = singles.tile([128, 2, K], fp8, tag="p8t")
            nc.sync.dma_start(
                out=p8t_sb, in_=p8t_d.rearrange("(c p) k -> p c k", p=128)
            )
            aug1l_sb = singles.tile([3, K], bf16, tag="aug1l")
            nc.sync.dma_start(out=aug1l_sb, in_=aug1l_d[:, :])
            v2aug_sb = singles.tile([3, T], bf16, tag="v2aug")
            nc.sync.dma_start(out=v2aug_sb, in_=v2aug_d[:, :])
            ones_col = singles.tile([128, 1], bf16, tag="ones_col")
            nc.sync.dma_start(out=ones_col, in_=consts_d[:, :])
            onesrow_sb = singles.tile([1, 128], bf16, tag="onesrow")
            nc.sync.dma_start(out=onesrow_sb, in_=onesrow_d[:, :])
            pn05_sb = []
            for kc in range(4):
                t = singles.tile([128, D], bf16, tag=f"pn05_{kc}")
                nc.sync.dma_start(out=t, in_=pn05_d[kc * 128 : (kc + 1) * 128, :])
                pn05_sb.append(t)
            tb = singles.tile([128, 2, tchunk], bf16, tag="xq_0")
            nc.sync.dma_start(
                out=tb, in_=xh_d[:, 0:tchunk].rearrange("(c p) t -> p c t", p=128)
            )
            xq.append(tb)
            ptm2_sb = []
            for d2 in range(2):
                t2 = singles.tile([128, K], bf16, tag=f"ptm2_{d2}")
                nc.sync.dma_start(out=t2, in_=ptm2_d[d2 * 128 : (d2 + 1) * 128, :])
                ptm2_sb.append(t2)
            aug2r_sb = singles.tile([2, K], bf16, tag="aug2r")
            nc.sync.dma_start(out=aug2r_sb, in_=aug2r_d[:, :])
            aug2l_sb = []
            for e in range(2):
                t = singles.tile([2, MACRO], bf16, tag=f"aug2l_{e}")
                nc.sync.dma_start(out=t, in_=rowinit_d[:, :])
                aug2l_sb.append(t)
            for cchunk in range(1, NCHUNK):
                t8 = singles.tile([128, 2, tchunk], fp8, tag=f"xq8_{cchunk}")
                nc.sync.dma_start(
                    out=t8,
                    in_=xh8_d[:, cchunk * tchunk : (cchunk + 1) * tchunk].rearrange(
                        "(c p) t -> p c t", p=128
                    ),
                )
                xq8.append(t8)
                tb = singles.tile([128, 2, tchunk], bf16, tag=f"xq_{cchunk}")
                nc.sync.dma_start(
                    out=tb,
                    in_=xh_d[:, cchunk * tchunk : (cchunk + 1) * tchunk].rearrange(
                        "(c p) t -> p c t", p=128
                    ),
                )
                xq.append(tb)

            state = {}

            def emit_pass1(im):
                tok0 = im * MACRO
                cchunk, coff = tok0 // tchunk, tok0 % tchunk
                ev = im % 2
                vth8 = xq8[cchunk][:, :, coff : coff + MACRO]
                vth = [xq[cchunk][:, d2, coff : coff + MACRO] for d2 in range(2)]
                v2slice = v2aug_sb[:, tok0 : tok0 + MACRO]
                # ---- dist1 (DoubleRow fp8 main + bf16 aug) + w1 ----
                wt = []
                for kc in range(4):
                    sqp = sq1_ps.tile([128, MACRO], f32, tag="sq1")
                    nc.tensor.matmul(
                        sqp,
                        p8t_sb[:, :, kc * 128 : (kc + 1) * 128],
                        vth8,
                        start=True,
                        stop=False,
                        perf_mode=DR,
                    )
                    nc.tensor.matmul(
                        sqp,
                        aug1l_sb[:, kc * 128 : (kc + 1) * 128],
                        v2slice,
                        start=False,
                        stop=True,
                    )
                    w = w1p.tile([128, MACRO], bf16, tag="wt")
                    recip_fast(nc, w, sqp)
                    wt.append(w)
                # ---- s1 row = sum_k w1 ----
                rows = rows_ps.tile([33, MACRO], f32, tag="rows")
                for kc in range(4):
                    nc.tensor.matmul(
                        rows[0:1, :],
                        ones_col,
                        wt[kc],
                        start=(kc == 0),
                        stop=(kc == 3),
                    )
                # ---- target^T: tg = 0.5 * protos^T @ w1 ----
                tg = []
                for d2 in range(2):
                    ps = tg_ps.tile([128, MACRO], f32, tag="tg")
                    for kc in range(4):
                        nc.tensor.matmul(
                            ps,
                            pn05_sb[kc][:, d2 * 128 : (d2 + 1) * 128],
                            wt[kc],
                            start=(kc == 0),
                            stop=(kc == 3),
                        )
                    tg.append(ps)
                isn = isnp.tile([1, MACRO], bf16, tag="isn")
                recip_fast(nc, isn, rows[0:1, :])
                bcq = bc_ps.tile([128, MACRO], f32, tag="bcq")
                nc.tensor.matmul(bcq, onesrow_sb, isn, start=True, stop=True)
                bcs = bcsp.tile([128, MACRO], bf16, tag="bcs")
                nc.scalar.copy(out=bcs, in_=bcq)
                state[im] = [None, rows, ev, tok0, tg, bcs, vth]

            def emit_x2asm(im):
                # emitted AFTER pass2(im-1): pass2's DVE ops fill the
                # isn->bc->bcs round-trip instead of stalling behind th/xt
                st = state[im]
                _, rows, ev, tok0, tg, bcs, vth = st
                xt = []
                for d2 in range(2):
                    th = thp.tile([128, MACRO], bf16, tag="th")
                    nc.vector.tensor_mul(th, tg[d2], bcs)
                    xtt = xtp.tile([128, MACRO], bf16, tag="xt")
                    nc.vector.tensor_add(xtt, th, vth[d2])
                    xt.append(xtt)
                st[0] = xt

            def emit_x2norm(im):
                xt, rows, ev, _ = state[im][:4]
                for d2 in range(2):
                    sq = sqxp.tile([128, MACRO], bf16, tag="sqx")
                    nc.scalar.square(sq, xt[d2])
                    nc.tensor.matmul(
                        rows[32:33, :],
                        ones_col,
                        sq,
                        start=(d2 == 0),
                        stop=(d2 == 1),
                    )
                nc.scalar.copy(out=aug2l_sb[ev][0:1, :], in_=rows[32:33, :])

            def emit_pass2(im):
                xt, rows, ev, tok0 = state.pop(im)[:4]
                ob4 = obp.tile([128, 4, K], f32, tag="ob")
                s2c = s2cp.tile([128, 4], f32, tag="s2c")
                r2 = r2p.tile([128, 4], f32, tag="r2")
                w2t = []
                for s in range(4):
                    ps2 = sq2_ps.tile([128, K], f32, tag="sq2")
                    for d2 in range(2):
                        nc.tensor.matmul(
                            ps2,
                            xt[d2][:, s * 128 : (s + 1) * 128],
                            ptm2_sb[d2],
                            start=(d2 == 0),
                            stop=False,
                        )
                    nc.tensor.matmul(
                        ps2,
                        aug2l_sb[ev][:, s * 128 : (s + 1) * 128],
                        aug2r_sb,
                        start=False,
                        stop=True,
                    )
                    w2 = w2p.tile([128, K], f32, tag="w2")
                    recip_fast(nc, w2, ps2)
                    w2t.append(w2)
                    scr = scrp.tile([128, K], bf16, tag="scr")
                    nc.scalar.activation(
                        out=scr, in_=w2, func=FT.Copy, accum_out=s2c[:, s : s + 1]
                    )
                recip_fast(nc, r2, s2c)
                for s in range(4):
                    if s % 2 == 0:
                        nc.scalar.mul(
                            out=ob4[:, s, :], in_=w2t[s], mul=r2[:, s : s + 1]
                        )
                    else:
                        nc.vector.tensor_scalar_mul(
                            ob4[:, s, :], w2t[s], r2[:, s : s + 1]
                        )
                nc.sync.dma_start(
                    out=out_d[tok0 : tok0 + MACRO, :].rearrange(
                        "(s p) k -> p s k", p=128
                    ),
                    in_=ob4,
                )

            for im in range(nmacro + 1):
                if im < nmacro:
                    emit_pass1(im)
                if im >= 1:
                    emit_pass2(im - 1)
                if im < nmacro:
                    emit_x2asm(im)
                    emit_x2norm(im)
    if do_compile:
        nc.compile()
    return nc


def static_inputs(protos):
    import ml_dtypes

    b = ml_dtypes.bfloat16
    f8 = ml_dtypes.float8_e4m3
    protos = np.ascontiguousarray(protos, dtype=np.float32)
    pt = protos.T  # [D, K]
    c2 = (protos * protos).sum(axis=1).astype(np.float32)  # [K]
    ones_k = np.ones(K, np.float32)
    # dist1 runs at scale 2 (lhsT=-8p^T fp8, rhs=0.5x fp8 -> -4xp = 2*(-2xp))
    aug1l = np.stack([ones_k, ones_k, 2.0 * c2])
    aug2r = np.stack([ones_k, c2])
    rowinit = np.stack([np.zeros(MACRO, np.float32), np.ones(MACRO, np.float32)])
    consts = np.ones((128, 1), np.float32)
    onesrow = np.ones((1, 128), np.float32)
    return {
        "aug1l": np.ascontiguousarray(aug1l).astype(b),
        "p8t": np.ascontiguousarray(-8.0 * pt).astype(f8),
        "ptm2": np.ascontiguousarray(-2.0 * pt).astype(b),
        "pn05": np.ascontiguousarray(0.5 * protos).astype(b),
        "aug2r": np.ascontiguousarray(aug2r).astype(b),
        "rowinit": np.ascontiguousarray(rowinit).astype(b),
        "consts": consts.astype(b),
        "onesrow": onesrow.astype(b),
    }


_NC_CACHE = {}


def _get_nc(T):
    if T not in _NC_CACHE:
        _NC_CACHE[T] = build_bass(T)
    return _NC_CACHE[T]


def _run(encodedData, protos, trace=False):
    import ml_dtypes
    from concourse.bass_utils import run_bass_kernel_spmd

    b = ml_dtypes.bfloat16
    f8 = ml_dtypes.float8_e4m3
    enc = np.ascontiguousarray(np.asarray(encodedData, dtype=np.float32))
    assert enc.shape == (B, N, D)
    T = (B // NCORES) * N
    nc = _get_nc(T)
    statics = static_inputs(np.asarray(protos, dtype=np.float32))
    bloc = B // NCORES
    in_maps = []
    for c in range(NCORES):
        xc = enc[c * bloc : (c + 1) * bloc].reshape(T, D)
        xh = np.ascontiguousarray(0.5 * xc.T)  # [D, T] f32
        v2 = 2.0 * (xc * xc).sum(axis=1).astype(np.float32)  # [T], dist1 scale 2
        v2hi = v2.astype(b)
        v2lo = (v2 - v2hi.astype(np.float32)).astype(b)
        v2aug = np.stack([v2hi, v2lo, np.ones(T, b)]).astype(b)
        in_maps.append(
            {
                "xh": xh.astype(b),
                "xh8": xh.astype(f8),
                "v2aug": np.ascontiguousarray(v2aug),
                **statics,
            }
        )
    res = run_bass_kernel_spmd(nc, in_maps, core_ids=list(range(NCORES)), trace=trace)
    out = np.empty((B, N, K), np.float32)
    for c in range(NCORES):
        out[c * bloc : (c + 1) * bloc] = res.results[c]["out"].reshape(bloc, N, K)
    return out, res


def kernel(**inputs):
    out, _ = _run(inputs["encodedData"], inputs["protos"])
    return out


def kernel_profiled(**inputs):
    out, res = _run(inputs["encodedData"], inputs["protos"], trace=True)
    return out, res


# revision 13
# speedup vs baseline: 1.0400x; 1.0400x over previous
"""Trainium2 Bass kernel for nn_CPCModel_50878182588587 (vq_codebook).

Computes, for inputs encodedData [B,N,D] and protos [K,D]:
  pass1: FCM memberships of v vs protos (p=2), x2 = 0.5*v + 0.5*(belong@protos)
  pass2: FCM memberships of x2 vs protos (p=2)  -> output [B,N,K]

Sharding: data-parallel over B across 8 NeuronCores; protos replicated.

v6 dataflow (T=8192 tokens/core, macro-tiles of 512 tokens):
  - Host pre-transposes the input: xh = 0.5*x^T in bf16 (x2 add, loaded in
    4 token-chunks so compute starts early) and fp8-e4m3 (dist1 matmuls);
    2*||x||^2 (hi/lo bf16 split) is a static row.
  - dist1 mains run as ONE fp8 DoubleRow matmul per k-chunk: the [128,2,*]
    chunk layout maps the 256-deep d-contraction onto a single matmul
    (out += sum_j lhsT[:,j].T @ rhs[:,j]); a 3-row bf16 augmented matmul
    adds 2*||v||^2 (hi/lo) + 2*c2.  The x2 distance scale cancels in the
    membership normalization.
  - dist2 stays bf16 (2 mains + 2-row aug) for accuracy: x2 feeds it as
    bf16 stationary slices.
  - 1/sq via single-op DVE reciprocal_approx_fast; s1-row via ones-matmul;
    isn broadcast via rank-1 matmul; x2 = vth + tg*isn on DVE.
  - pass2 in orientation A: w2=1/sq2 (f32) on DVE; s2 via ACT copy+accum;
    final normalize split 2/2 between ACT (per-partition scale column) and
    DVE tensor_scalar (NO gpsimd -- its tensor_scalar measured 7.5us/tile).
  - Emission is software-pipelined: pass2(i-1) sits between pass1(i) and
    x2norm(i) so the PE stream never starves (keeps HAM warm).
"""

import sys

import numpy as np

sys.path.insert(0, "/opt/trn_rl_repo")

import concourse.bass as bass  # noqa: E402
from concourse import bacc  # noqa: E402
import concourse.mybir as mybir  # noqa: E402
import concourse.tile as tile  # noqa: E402

B, N, D, K = 64, 1024, 256, 512
NCORES = 8
MACRO = 512  # tokens per macro-tile
NCHUNK = 8  # input token-chunks (startup latency)
f32 = mybir.dt.float32
bf16 = mybir.dt.bfloat16
fp8 = mybir.dt.float8e4
FT = mybir.ActivationFunctionType
OP = mybir.AluOpType
DR = mybir.MatmulPerfMode.DoubleRow


def recip_fast(nc, out, in_):
    """reciprocal_approx_fast with any output dtype (wrapper asserts fp32)."""
    from concourse.dve_ops import RECIP_APPROX_FAST_CONSTS, RECIPROCAL_APPROX_FAST

    c = RECIP_APPROX_FAST_CONSTS
    return nc.vector._custom_dve(
        RECIPROCAL_APPROX_FAST, out=out, in0=in_, s0=c["s0"], s1=c["s1"], imm2=c["imm2"]
    )


def build_bass(T, do_compile=True):
    assert T % MACRO == 0 and T % NCHUNK == 0
    nmacro = T // MACRO
    tchunk = T // NCHUNK
    nc = bacc.Bacc(trn_type="TRN2")

    # xh layouts [2*128, T]: chunk c holds x^T rows c*128..c*128+127 (x 0.5)
    xh_d = nc.dram_tensor("xh", [2 * 128, T], bf16, kind="ExternalInput")
    xh8_d = nc.dram_tensor("xh8", [2 * 128, T], fp8, kind="ExternalInput")
    # v2aug rows: [0]=v2hi, [1]=v2lo (hi/lo split of 2*||x||^2), [2]=ones
    v2aug_d = nc.dram_tensor("v2aug", [3, T], bf16, kind="ExternalInput")
    # aug1l rows: [0]=ones, [1]=ones, [2]=2*c2   (pairs with v2aug rows)
    aug1l_d = nc.dram_tensor("aug1l", [3, K], bf16, kind="ExternalInput")
    # dist1 DoubleRow stationary [128, 2, K]: [:,j,:] = chunk j of -8*protos.T
    p8t_d = nc.dram_tensor("p8t", [2 * 128, K], fp8, kind="ExternalInput")
    ptm2_d = nc.dram_tensor("ptm2", [D, K], bf16, kind="ExternalInput")  # -2*protos.T
    pn05_d = nc.dram_tensor("pn05", [K, D], bf16, kind="ExternalInput")  # 0.5*protos
    # aug2r rows: [0]=ones (pairs x2n), [1]=c2
    aug2r_d = nc.dram_tensor("aug2r", [2, K], bf16, kind="ExternalInput")
    rowinit_d = nc.dram_tensor("rowinit", [2, MACRO], bf16, kind="ExternalInput")
    consts_d = nc.dram_tensor("consts", [128, 1], bf16, kind="ExternalInput")  # ones
    onesrow_d = nc.dram_tensor("onesrow", [1, 128], bf16, kind="ExternalInput")
    out_d = nc.dram_tensor("out", [T, K], f32, kind="ExternalOutput")

    with tile.TileContext(nc) as tc:
        with (
            tc.tile_pool(name="singles", bufs=1) as singles,
            tc.tile_pool(name="w1", bufs=8) as w1p,
            tc.tile_pool(name="isn", bufs=2) as isnp,
            tc.tile_pool(name="bcs", bufs=2) as bcsp,
            tc.tile_pool(name="th", bufs=4) as thp,
            tc.tile_pool(name="xt", bufs=4) as xtp,
            tc.tile_pool(name="sqx", bufs=4) as sqxp,
            tc.tile_pool(name="w2", bufs=8) as w2p,
            tc.tile_pool(name="scr", bufs=2) as scrp,
            tc.tile_pool(name="s2c", bufs=2) as s2cp,
            tc.tile_pool(name="r2", bufs=2) as r2p,
            tc.tile_pool(name="ob", bufs=2) as obp,
            tc.tile_pool(name="sq1p", bufs=2, space="PSUM") as sq1_ps,
            tc.tile_pool(name="rwp", bufs=1, space="PSUM") as rows_ps,
            tc.tile_pool(name="bcp", bufs=1, space="PSUM") as bc_ps,
            tc.tile_pool(name="tgp", bufs=2, space="PSUM") as tg_ps,
            tc.tile_pool(name="sq2p", bufs=2, space="PSUM") as sq2_ps,
        ):
            # ---- statics, ordered so macro 0's dependencies land first ----
            xq8 = []
            xq = []
            t8 = singles.tile([128, 2, tchunk], fp8, tag="xq8_0")
            nc.sync.dma_start(
                out=t8, in_=xh8_d[:, 0:tchunk].rearrange("(c p) t -> p c t", p=128)
            )
            xq8.append(t8)
            p8t_sb = singles.tile([128, 2, K], fp8, tag="p8t")
            nc.sync.dma_start(
                out=p8t_sb, in_=p8t_d.rearrange("(c p) k -> p c k", p=128)
            )
            aug1l_sb = singles.tile([3, K], bf16, tag="aug1l")
            nc.sync.dma_start(out=aug1l_sb, in_=aug1l_d[:, :])
            v2aug_sb = singles.tile([3, T], bf16, tag="v2aug")
            nc.sync.dma_start(out=v2aug_sb, in_=v2aug_d[:, :])
            ones_col = singles.tile([128, 1], bf16, tag="ones_col")
            nc.sync.dma_start(out=ones_col, in_=consts_d[:, :])
            onesrow_sb = singles.tile([1, 128], bf16, tag="onesrow")
            nc.sync.dma_start(out=onesrow_sb, in_=onesrow_d[:, :])
            pn05_sb = []
            for kc in range(4):
                t = singles.tile([128, D], bf16, tag=f"pn05_{kc}")
                nc.sync.dma_start(out=t, in_=pn05_d[kc * 128 : (kc + 1) * 128, :])
                pn05_sb.append(t)
            tb = singles.tile([128, 2, tchunk], bf16, tag="xq_0")
            nc.sync.dma_start(
                out=tb, in_=xh_d[:, 0:tchunk].rearrange("(c p) t -> p c t", p=128)
            )
            xq.append(tb)
            ptm2_sb = []
            for d2 in range(2):
                t2 = singles.tile([128, K], bf16, tag=f"ptm2_{d2}")
                nc.sync.dma_start(out=t2, in_=ptm2_d[d2 * 128 : (d2 + 1) * 128, :])
                ptm2_sb.append(t2)
            aug2r_sb = singles.tile([2, K], bf16, tag="aug2r")
            nc.sync.dma_start(out=aug2r_sb, in_=aug2r_d[:, :])
            aug2l_sb = []
            for e in range(2):
                t = singles.tile([2, MACRO], bf16, tag=f"aug2l_{e}")
                nc.sync.dma_start(out=t, in_=rowinit_d[:, :])
                aug2l_sb.append(t)
            for cchunk in range(1, NCHUNK):
                t8 = singles.tile([128, 2, tchunk], fp8, tag=f"xq8_{cchunk}")
                nc.sync.dma_start(
                    out=t8,
                    in_=xh8_d[:, cchunk * tchunk : (cchunk + 1) * tchunk].rearrange(
                        "(c p) t -> p c t", p=128
                    ),
                )
                xq8.append(t8)
                tb = singles.tile([128, 2, tchunk], bf16, tag=f"xq_{cchunk}")
                nc.sync.dma_start(
                    out=tb,
                    in_=xh_d[:, cchunk * tchunk : (cchunk + 1) * tchunk].rearrange(
                        "(c p) t -> p c t", p=128
                    ),
                )
                xq.append(tb)

            state = {}

            def emit_pass1(im):
                tok0 = im * MACRO
                cchunk, coff = tok0 // tchunk, tok0 % tchunk
                ev = im % 2
                vth8 = xq8[cchunk][:, :, coff : coff + MACRO]
                vth = [xq[cchunk][:, d2, coff : coff + MACRO] for d2 in range(2)]
                v2slice = v2aug_sb[:, tok0 : tok0 + MACRO]
                # ---- dist1 (DoubleRow fp8 main + bf16 aug) + w1 ----
                wt = []
                for kc in range(4):
                    sqp = sq1_ps.tile([128, MACRO], f32, tag="sq1")
                    nc.tensor.matmul(
                        sqp,
                        p8t_sb[:, :, kc * 128 : (kc + 1) * 128],
                        vth8,
                        start=True,
                        stop=False,
                        perf_mode=DR,
                    )
                    nc.tensor.matmul(
                        sqp,
                        aug1l_sb[:, kc * 128 : (kc + 1) * 128],
                        v2slice,
                        start=False,
                        stop=True,
                    )
                    w = w1p.tile([128, MACRO], bf16, tag="wt")
                    recip_fast(nc, w, sqp)
                    wt.append(w)
                # ---- s1 row = sum_k w1 ----
                rows = rows_ps.tile([33, MACRO], f32, tag="rows")
                for kc in range(4):
                    nc.tensor.matmul(
                        rows[0:1, :],
                        ones_col,
                        wt[kc],
                        start=(kc == 0),
                        stop=(kc == 3),
                    )
                # ---- target^T: tg = 0.5 * protos^T @ w1 ----
                tg = []
                for d2 in range(2):
                    ps = tg_ps.tile([128, MACRO], f32, tag="tg")
                    for kc in range(4):
                        nc.tensor.matmul(
                            ps,
                            pn05_sb[kc][:, d2 * 128 : (d2 + 1) * 128],
                            wt[kc],
                            start=(kc == 0),
                            stop=(kc == 3),
                        )
                    tg.append(ps)
                isn = isnp.tile([1, MACRO], bf16, tag="isn")
                recip_fast(nc, isn, rows[0:1, :])
                bcq = bc_ps.tile([128, MACRO], f32, tag="bcq")
                nc.tensor.matmul(bcq, onesrow_sb, isn, start=True, stop=True)
                bcs = bcsp.tile([128, MACRO], bf16, tag="bcs")
                nc.scalar.copy(out=bcs, in_=bcq)
                # ---- x2^T = 0.5 v^T + isn * tg ----
                xt = []
                for d2 in range(2):
                    th = thp.tile([128, MACRO], bf16, tag="th")
                    nc.vector.tensor_mul(th, tg[d2], bcs)
                    xtt = xtp.tile([128, MACRO], bf16, tag="xt")
                    nc.vector.tensor_add(xtt, th, vth[d2])
                    xt.append(xtt)
                state[im] = (xt, rows, ev, tok0)

            def emit_x2norm(im):
                xt, rows, ev, _ = state[im]
                for d2 in range(2):
                    sq = sqxp.tile([128, MACRO], bf16, tag="sqx")
                    nc.scalar.square(sq, xt[d2])
                    nc.tensor.matmul(
                        rows[32:33, :],
                        ones_col,
                        sq,
                        start=(d2 == 0),
                        stop=(d2 == 1),
                    )
                nc.scalar.copy(out=aug2l_sb[ev][0:1, :], in_=rows[32:33, :])

            def emit_pass2(im):
                xt, rows, ev, tok0 = state.pop(im)
                ob4 = obp.tile([128, 4, K], f32, tag="ob")
                s2c = s2cp.tile([128, 4], f32, tag="s2c")
                r2 = r2p.tile([128, 4], f32, tag="r2")
                w2t = []
                for s in range(4):
                    ps2 = sq2_ps.tile([128, K], f32, tag="sq2")
                    for d2 in range(2):
                        nc.tensor.matmul(
                            ps2,
                            xt[d2][:, s * 128 : (s + 1) * 128],
                            ptm2_sb[d2],
                            start=(d2 == 0),
                            stop=False,
                        )
                    nc.tensor.matmul(
                        ps2,
                        aug2l_sb[ev][:, s * 128 : (s + 1) * 128],
                        aug2r_sb,
                        start=False,
                        stop=True,
                    )
                    w2 = w2p.tile([128, K], f32, tag="w2")
                    recip_fast(nc, w2, ps2)
                    w2t.append(w2)
                    scr = scrp.tile([128, K], bf16, tag="scr")
                    nc.scalar.activation(
                        out=scr, in_=w2, func=FT.Copy, accum_out=s2c[:, s : s + 1]
                    )
                recip_fast(nc, r2, s2c)
                for s in range(4):
                    if s % 2 == 0:
                        nc.scalar.mul(
                            out=ob4[:, s, :], in_=w2t[s], mul=r2[:, s : s + 1]
                        )
                    else:
                        nc.vector.tensor_scalar_mul(
                            ob4[:, s, :], w2t[s], r2[:, s : s + 1]
                        )
                nc.sync.dma_start(
                    out=out_d[tok0 : tok0 + MACRO, :].rearrange(
                        "(s p) k -> p s k", p=128
                    ),
                    in_=ob4,
                )

            for im in range(nmacro + 1):
                if im < nmacro:
                    emit_pass1(im)
                if im >= 1:
                    emit_pass2(im - 1)
                if im < nmacro:
                    emit_x2norm(im)
    if do_compile:
        nc.compile()
    return nc


def static_inputs(protos):
    import ml_dtypes

    b = ml_dtypes.bfloat16
    f8 = ml_dtypes.float8_e4m3
    protos = np.ascontiguousarray(protos, dtype=np.float32)
    pt = protos.T  # [D, K]
    c2 = (protos * protos).sum(axis=1).astype(np.float32)  # [K]
    ones_k = np.ones(K, np.float32)
    # dist1 runs at scale 2 (lhsT=-8p^T fp8, rhs=0.5x fp8 -> -4xp = 2*(-2xp))
    aug1l = np.stack([ones_k, ones_k, 2.0 * c2])
    aug2r = np.stack([ones_k, c2])
    rowinit = np.stack([np.zeros(MACRO, np.float32), np.ones(MACRO, np.float32)])
    consts = np.ones((128, 1), np.float32)
    onesrow = np.ones((1, 128), np.float32)
    return {
        "aug1l": np.ascontiguousarray(aug1l).astype(b),
        "p8t": np.ascontiguousarray(-8.0 * pt).astype(f8),
        "ptm2": np.ascontiguousarray(-2.0 * pt).astype(b),
        "pn05": np.ascontiguousarray(0.5 * protos).astype(b),
        "aug2r": np.ascontiguousarray(aug2r).astype(b),
        "rowinit": np.ascontiguousarray(rowinit).astype(b),
        "consts": consts.astype(b),
        "onesrow": onesrow.astype(b),
    }


_NC_CACHE = {}


def _get_nc(T):
    if T not in _NC_CACHE:
        _NC_CACHE[T] = build_bass(T)
    return _NC_CACHE[T]


def _run(encodedData, protos, trace=False):
    import ml_dtypes
    from concourse.bass_utils import run_bass_kernel_spmd

    b = ml_dtypes.bfloat16
    f8 = ml_dtypes.float8_e4m3
    enc = np.ascontiguousarray(np.asarray(encodedData, dtype=np.float32))
    assert enc.shape == (B, N, D)
    T = (B // NCORES) * N
    nc = _get_nc(T)
    statics = static_inputs(np.asarray(protos, dtype=np.float32))
    bloc = B // NCORES
    in_maps = []
    for c in range(NCORES):
        xc = enc[c * bloc : (c + 1) * bloc].reshape(T, D)
        xh = np.ascontiguousarray(0.5 * xc.T)  # [D, T] f32
        v2 = 2.0 * (xc * xc).sum(axis=1).astype(np.float32)  # [T], dist1 scale 2
        v2hi = v2.astype(b)
        v2lo = (v2 - v2hi.astype(np.float32)).astype(b)
        v2aug = np.stack([v2hi, v2lo, np.ones(T, b)]).astype(b)
        in_maps.append(
            {
                "xh": xh.astype(b),
                "xh8": xh.astype(f8),
                "v2aug": np.ascontiguousarray(v2aug),
                **statics,
            }
        )
    res = run_bass_kernel_spmd(nc, in_maps, core_ids=list(range(NCORES)), trace=trace)
    out = np.empty((B, N, K), np.float32)
    for c in range(NCORES):
        out[c * bloc : (c + 1) * bloc] = res.results[c]["out"].reshape(bloc, N, K)
    return out, res


def kernel(**inputs):
    out, _ = _run(inputs["encodedData"], inputs["protos"])
    return out


def kernel_profiled(**inputs):
    out, res = _run(inputs["encodedData"], inputs["protos"], trace=True)
    return out, res
